# revision 11
# baseline (speedup 1.0000x reference)
"""CIF (continuous integrate-and-fire) kernel for Trainium2, 8 NeuronCores.

Strategy (pure data parallel over batch, one batch row per core):
  - Host computes the scalar weight row w = sigmoid(feat @ W + b) (16 MFLOP,
    negligible next to the 32 GFLOP aggregation) and runs the inherently
    sequential integrate-and-fire scan in float32, reproducing the reference
    recurrence bit-for-bit.  This yields, per frame t: its output bucket k_t
    and its (at most two) assignment coefficients.
  - The heavy, memory-bound work runs on device: each core loads its 4.2 MB
    feat row, multiplies it against banded 128x128 coefficient tiles on the
    tensor engine (the assignment matrix is a monotone band: a 128-frame
    chunk only ever touches ~66 consecutive output rows), accumulating each
    128-row output block in PSUM, and DMAs PSUM straight back to HBM.
  - Output rows beyond the last fired bucket are exactly zero and are not
    computed or transferred; the host pads them.
"""

import os
import sys

import numpy as np

if "/opt/trn_rl_repo" not in sys.path and not any(
    os.path.isdir(os.path.join(p, "concourse")) for p in sys.path if p
):
    sys.path.insert(0, "/opt/trn_rl_repo")

B, T, D = 8, 2000, 512
TP = 2048            # t padded to a multiple of 128
TCH = 128            # contraction chunk (t frames per matmul)
NCHUNK = TP // TCH   # 16
MBLK = 128           # output rows per PSUM block

_f32 = np.float32

LAST_RESULTS = None  # BassKernelResults of the most recent device run


def _host_weight(feat, W, b):
    """w = sigmoid(feat @ W + b[0]) and hlens_new, matching the reference.

    Prefer jax on CPU so the einsum/sigmoid/sum rounding matches the
    reference implementation exactly; fall back to numpy.
    """
    try:
        import jax
        import jax.numpy as jnp

        cpu = jax.devices("cpu")[0]
        with jax.default_device(cpu):
            w = jax.nn.sigmoid(jnp.einsum("btd,d->bt", feat, W) + b[0])
            hlens_new = jnp.ceil(jnp.sum(w, axis=1)).astype(jnp.int32)
            return np.asarray(w, dtype=_f32), np.asarray(hlens_new, dtype=np.int32)
    except Exception:
        w = feat.reshape(-1, D).astype(_f32) @ W.astype(_f32)
        w = w.reshape(B, T) + _f32(b[0])
        w = _f32(1.0) / (_f32(1.0) + np.exp(-w, dtype=_f32))
        hlens_new = np.ceil(np.sum(w, axis=1, dtype=_f32)).astype(np.int32)
        return w.astype(_f32), hlens_new


def _host_scan(w):
    """Integrate-and-fire scan in float32, replicating the reference exactly.

    Returns (ks, c0, c1): for frame t of row b,
      ks[b,t] = bucket index k before processing t
      c0[b,t] = weight_mat contribution of frame t to bucket k
      c1[b,t] = contribution to bucket k+1 (0 unless the frame fires)
    """
    one = _f32(1.0)
    zero = _f32(0.0)
    acc = np.zeros(B, _f32)
    k = np.zeros(B, np.int64)
    ks = np.empty((B, T), np.int32)
    c0 = np.empty((B, T), _f32)
    c1 = np.zeros((B, T), _f32)
    for t in range(T):
        wt = w[:, t]
        s = acc + wt                      # f32
        fire = s > one
        ks[:, t] = k
        # reference: coef = w * ((1-acc)/w) and w * ((acc+w-1)/w)
        c0[:, t] = np.where(fire, wt * ((one - acc) / wt), wt)
        c1[:, t] = np.where(fire, wt * ((s - one) / wt), zero)
        acc = np.where(fire, zero, s)
        k = k + fire
    return ks, c0, c1


def _build_structure(ks, c0, c1):
    """Static (core-independent) banded-matmul structure + per-core tiles.

    Returns (q_rows, chunk_list, coef) where
      q_rows: number of 128-row output blocks Q
      chunk_list: list of (q, n) pairs, in emission order
      coef: [B, len(chunk_list), 128, 128] float32 lhsT tiles
    """
    fire = c1 != 0
    touched_hi = ks + fire  # k+1 when the frame fires
    kmax = int(touched_hi.max())
    q_rows = (kmax + 1 + MBLK - 1) // MBLK

    qlo = np.minimum(ks // MBLK, touched_hi // MBLK)   # [B, T]
    qhi = np.maximum(ks // MBLK, touched_hi // MBLK)

    chunk_list = []
    tiles = []
    t_idx = np.arange(T)
    for q in range(q_rows):
        mask = (qlo == q) | (qhi == q)          # [B, T]
        any_mask = mask.any(axis=0)
        ts = t_idx[any_mask]
        n_lo, n_hi = int(ts[0]) // TCH, int(ts[-1]) // TCH
        for n in range(n_lo, n_hi + 1):
            t0, t1 = n * TCH, min((n + 1) * TCH, T)
            sub = slice(t0, t1)
            if not mask[:, sub].any():
                continue
            A = np.zeros((B, TCH, MBLK), _f32)
            trel = np.arange(t1 - t0)
            for b in range(B):
                m0 = ks[b, sub] - q * MBLK
                v0 = np.logical_and(m0 >= 0, m0 < MBLK)
                A[b, trel[v0], m0[v0]] = c0[b, sub][v0]
                m1 = m0 + 1
                v1 = np.logical_and.reduce([m1 >= 0, m1 < MBLK, fire[b, sub]])
                A[b, trel[v1], m1[v1]] += c1[b, sub][v1]
            chunk_list.append((q, n))
            tiles.append(A)
    coef = np.stack(tiles, axis=1) if tiles else np.zeros((B, 0, TCH, MBLK), _f32)
    return q_rows, chunk_list, coef


def _round_fp32r(x):
    """Round fp32 to the float32r-representable set (bf16 hi + bf16 lo)."""

    def _to_bf16(v):
        u = v.view(np.uint32)
        u = (u + np.uint32(0x7FFF) + ((u >> np.uint32(16)) & np.uint32(1))) & np.uint32(
            0xFFFF0000
        )
        return u.view(np.float32)

    x = np.ascontiguousarray(x, dtype=np.float32)
    hi = _to_bf16(x)
    lo = _to_bf16((x - hi).astype(np.float32))
    return (hi + lo).astype(np.float32)


def _run_device(feat, coef, q_rows, chunk_list):
    """Build + run the bass kernel on 8 cores. Returns [B, Q*128, 512] f32."""
    import concourse.bacc as bacc
    import concourse.mybir as mybir
    from concourse import bass, tile
    from concourse.bass_utils import run_bass_kernel_spmd

    global LAST_RESULTS

    nch = len(chunk_list)
    f32 = mybir.dt.float32
    f32r = mybir.dt.float32r

    nc = bacc.Bacc("TRN2", target_bir_lowering=False, debug=False)
    feat_d = nc.dram_tensor("feat", [TP, D], f32r, kind="ExternalInput")
    coef_d = nc.dram_tensor("coef", [nch * TCH, MBLK], f32r, kind="ExternalInput")
    out_d = nc.dram_tensor("out", [q_rows * MBLK, D], f32, kind="ExternalOutput")

    # chunks in first-use order so feat DMAs can stream ahead of the PE
    used_chunks = []
    for q, n in chunk_list:
        if n not in used_chunks:
            used_chunks.append(n)

    with tile.TileContext(nc) as tc:
        with (
            tc.tile_pool(name="coefp", bufs=1) as coefp,
            tc.tile_pool(name="featp", bufs=1) as featp,
            tc.tile_pool(name="psump", bufs=4, space=bass.MemorySpace.PSUM) as psump,
            tc.tile_pool(name="outp", bufs=3) as outp,
        ):
            coef_t = coefp.tile([TCH, nch, MBLK], f32r)
            nc.sync.dma_start(
                coef_t[:], coef_d.ap().rearrange("(c p) m -> p c m", p=TCH)
            )
            feat_tiles = {}
            for n in used_chunks:
                ft = featp.tile([TCH, D], f32r, tag=f"feat{n}")
                nc.sync.dma_start(ft[:], feat_d.ap()[n * TCH : (n + 1) * TCH, :])
                feat_tiles[n] = ft

            by_q = {}
            for i, (q, n) in enumerate(chunk_list):
                by_q.setdefault(q, []).append((i, n))
            for q in range(q_rows):
                acc = psump.tile([MBLK, D], f32)
                items = by_q[q]
                for j, (i, n) in enumerate(items):
                    nc.tensor.matmul(
                        acc[:],
                        coef_t[:, i, :],
                        feat_tiles[n][:],
                        start=(j == 0),
                        stop=(j == len(items) - 1),
                    )
                ot = outp.tile([MBLK, D], f32, tag="ot")
                nc.scalar.copy(ot[:], acc[:])
                nc.sync.dma_start(out_d.ap()[q * MBLK : (q + 1) * MBLK, :], ot[:])

    nc.compile()

    feat_pad = np.zeros((B, TP, D), _f32)
    feat_pad[:, :T, :] = feat
    in_maps = [
        {
            "feat": _round_fp32r(feat_pad[b]),
            "coef": _round_fp32r(coef[b].reshape(nch * TCH, MBLK)),
        }
        for b in range(B)
    ]
    res = run_bass_kernel_spmd(nc, in_maps, core_ids=list(range(B)))
    LAST_RESULTS = res
    return np.stack([res.results[b]["out"] for b in range(B)], axis=0)


def kernel(feat, hlens, W, b):
    feat = np.asarray(feat, dtype=_f32)
    W = np.asarray(W, dtype=_f32)
    b = np.asarray(b, dtype=_f32)

    w, hlens_new = _host_weight(feat, W, b)
    ks, c0, c1 = _host_scan(w)
    q_rows, chunk_list, coef = _build_structure(ks, c0, c1)

    out_dev = _run_device(feat, coef, q_rows, chunk_list)

    feat_new = np.zeros((B, T + 1, D), _f32)
    rows = min(q_rows * MBLK, T + 1)
    feat_new[:, :rows, :] = out_dev[:, :rows, :]
    return feat_new, hlens_new


if __name__ == "__main__":
    rng = np.random.default_rng(0)
    feat = rng.standard_normal((B, T, D), dtype=_f32)
    hlens = rng.integers(0, T, size=(B,)).astype(np.int32)
    W = (rng.standard_normal(D, dtype=_f32) / np.sqrt(D)).astype(_f32)
    b = np.zeros(1, _f32)
    out, hl = kernel(feat, hlens, W, b)
    print(out.shape, out.dtype, hl)
